# revision 31
# baseline (speedup 1.0000x reference)
"""Trainium2 Bass kernel for DirectTargetLoss.

Computes, from sparse_rep [256, 128000] f32 and target_ids [256, 16] i64:
  target_loss   = -mean(log(gather(sparse_rep, target_ids) + 1e-8))
  margin_loss   = mean(relu(1 - gather(sparse_rep, target_ids)))
  negative_loss = mean(top_k(sparse_rep with target cols masked to -1e30, 100))

Sharding: data-parallel over the batch axis across 8 NeuronCores
(32 rows/core).  Per core:
  - the [32, 128000] shard is streamed into SBUF as 8 tiles of
    [128, 4000] f32 (each tile = 4 rows x 2 column-halves of 64000;
    each half spread over 16 partitions),
  - the gpsimd top-256 instruction runs per tile (8 "tokens" = 8
    half-rows; exact, sorted top-256 values per half),
  - per row, the top-128 of each half (sorted tails) are concatenated
    into a 256-candidate tile (the masked top-100 of the full row is
    always contained in these 240+ survivors),
  - the row's 16 target activations (gathered via 4 indirect DMAs of
    128 offsets each) are zapped out of the candidates by exact value
    match (match_replace); equal-value collisions can only swap equal
    contributions, so the top-100 sum is unchanged,
  - 13 rounds of max8 + match_replace extract the top 100 exactly,
  - the gathered targets also feed Ln / Relu activations with
    accumulation for the other two losses,
  - a ones-vector matmul reduces the per-row partials to [1, 3].
Host sums the 8 per-core [1,3] partials and normalizes.
"""

import numpy as np

B = 256
V = 128000
T = 16
TOP_K = 100
EPS = 1e-8
N_CORES = 8
BL = B // N_CORES          # 32 rows per core
RPT = 4                    # rows per topk call (2 halves each -> 8 tokens)
NT = BL // RPT             # 8 tiles per core
HALF = V // 2              # 64000
SEG = 16                   # partitions per half-row
F = HALF // SEG            # 4000 free elems per partition
KC = 256                   # topk k (per half-row)
TAIL = 128                 # kept tail per half
NC_CAND = 2 * TAIL         # 256 candidates per row
GW = T // 4                # 4 gather calls of 128 offsets

_CACHE = {}


def _build_nc(
    do_topk=True,
    do_cand=True,
    do_gather=True,
    load_engines=("sync",),
    loads_per_tile=1,
    cand_level=99,
    cand_rounds=13,
):
    from contextlib import ExitStack

    import concourse.bass as bass
    import concourse.tile as tile
    from concourse import bacc, mybir

    f32 = mybir.dt.float32
    i32 = mybir.dt.int32
    u32 = mybir.dt.uint32
    AF = mybir.ActivationFunctionType
    OP = mybir.AluOpType

    nc = bacc.Bacc("TRN2", target_bir_lowering=False, debug=False)

    sp = nc.dram_tensor("sp", [BL, V], f32, kind="ExternalInput")
    off = nc.dram_tensor("off", [128, GW], i32, kind="ExternalInput")
    out3 = nc.dram_tensor("out3", [1, 3], f32, kind="ExternalOutput")

    with tile.TileContext(nc) as tc, ExitStack() as ctx:
        cand_pool = ctx.enter_context(tc.tile_pool(name="cand", bufs=1))
        small_pool = ctx.enter_context(tc.tile_pool(name="small", bufs=1))
        psum_pool = ctx.enter_context(tc.tile_pool(name="psum", bufs=1, space="PSUM"))

        valsF = cand_pool.tile([BL, 2 * KC], f32, tag="valsF")
        b0 = cand_pool.tile([BL, TAIL], f32, tag="b0")
        b1 = cand_pool.tile([BL, TAIL], f32, tag="b1")
        zt = cand_pool.tile([BL, TAIL], f32, tag="zt")
        ct = cand_pool.tile([BL, TAIL], f32, tag="ct")
        zz = cand_pool.tile([BL, TAIL], f32, tag="zz")
        wt = cand_pool.tile([BL, TAIL], f32, tag="wt")
        pt = cand_pool.tile([BL, TAIL], f32, tag="pt")

        off_sb = small_pool.tile([128, GW], i32, tag="off_sb")
        tgtw = small_pool.tile([128, GW], f32, tag="tgtw")
        tgt32 = small_pool.tile([BL, T], f32, tag="tgt32")
        lnoutW = small_pool.tile([128, GW], f32, tag="lnoutW")
        mgoutW = small_pool.tile([128, GW], f32, tag="mgoutW")
        eps_t = small_pool.tile([128, 1], f32, tag="eps_t")
        thr = small_pool.tile([BL, 1], f32, tag="thr")
        negrow = small_pool.tile([BL, 1], f32, tag="negrow")
        stacked = small_pool.tile([128, 3], f32, tag="stacked")
        ones = small_pool.tile([128, 1], f32, tag="ones")
        out_sb = small_pool.tile([1, 3], f32, tag="out_sb")

        load_eng = [getattr(nc, e) for e in load_engines]

        nc.vector.memset(stacked[:], 0.0)
        nc.vector.memset(negrow[:], 0.0)

        # offsets for the gathers: off[p, g] targets (row p%32, t = 4*(p//32)+g)
        nc.sync.dma_start(off_sb[:], off[:, :])

        # --- big loads + per-half top-256 ---
        # tile c covers rows 4c..4c+3; token (j,h) = row 4c+j, col half h
        # sits on partitions 16*(2j+h) .. +15, 4000 contiguous elems each.
        dma_i = 0
        for c in range(NT):
            data = nc.alloc_sbuf_tensor(f"data{c}", [128, F], f32).ap()
            src = sp[RPT * c:RPT * (c + 1), :].rearrange(
                "j (h s f) -> (j h s) f", h=2, s=SEG
            )
            pchunk = 128 // loads_per_tile
            for li in range(loads_per_tile):
                eng = load_eng[dma_i % len(load_eng)]
                dma_i += 1
                pr = slice(pchunk * li, pchunk * (li + 1))
                eng.dma_start(data[pr, :], src[pr, :])

            if do_topk:
                tout = nc.alloc_sbuf_tensor(f"tout{c}", [128, 32], u32).ap()
                nc.gpsimd.topk(tout[:], data[:], tokens=8, vocab_size=HALF, k=KC)

                if do_cand:
                    # all 256 sorted-asc values per half -> row-major
                    # [4 rows, 2*256] (row r cols = [h0 asc | h1 asc])
                    dst_v = valsF[RPT * c:RPT * (c + 1), :].rearrange(
                        "j (h s f) -> j h s f", h=2, f=16
                    )
                    nc.scalar.dma_start(dst_v, tout[:, 0:16].bitcast(f32))

        # --- target gather: 4 indirect DMAs, 128 offsets each ---
        if do_gather:
            sp_flat = sp[:, :].rearrange("b (v one) -> (b v) one", one=1)
            for g in range(GW):
                nc.gpsimd.indirect_dma_start(
                    out=tgtw[:, g:g + 1],
                    out_offset=None,
                    in_=sp_flat,
                    in_offset=bass.IndirectOffsetOnAxis(
                        ap=off_sb[:, g:g + 1], axis=0
                    ),
                )
            # target_loss partial: sum(log(tgt + eps)); margin: sum(relu(1-tgt))
            nc.vector.memset(eps_t[:], EPS)
            nc.scalar.activation(
                lnoutW[:], tgtw[:], AF.Ln,
                bias=eps_t[:, 0:1], scale=1.0, accum_out=stacked[:, 0:1],
            )
            nc.scalar.activation(
                mgoutW[:], tgtw[:], AF.Relu,
                bias=1.0, scale=-1.0, accum_out=stacked[:, 1:2],
            )
            # row-major copy for the candidate zap: tgt32[r, 4q+g] =
            # tgtw[r+32q, g]
            for q in range(4):
                nc.scalar.dma_start(
                    tgt32[:, 4 * q:4 * (q + 1)], tgtw[32 * q:32 * (q + 1), :]
                )

        if do_topk and do_cand and cand_level >= 2:
            # --- top-128 of the union of the two sorted half-tails ---
            # A = half0 tail (asc), revB = half1 tail reversed (desc);
            # [A | revB] is bitonic, so hi_i = max(A_i, B_rev_i) is the
            # top-128 of the union (itself bitonic).
            a_view = valsF[:, KC - TAIL:KC]
            b_last = valsF[:, 2 * KC - 1:2 * KC]
            b_rev = bass.AP(
                b_last.tensor, b_last.offset,
                [list(b_last.ap[0]), [-1, TAIL]],
            )
            nc.vector.tensor_tensor(b0[:], a_view, b_rev, op=OP.max)

            # --- bitonic merge: 7 stages of paired min/max -> ascending ---
            cur, nxt = b0, b1
            d = TAIL // 2
            while d >= 1:
                cv = cur[:].rearrange("p (b two d) -> p b two d", two=2, d=d)
                nv = nxt[:].rearrange("p (b two d) -> p b two d", two=2, d=d)
                nc.vector.tensor_tensor(
                    nv[:, :, 0, :], cv[:, :, 0, :], cv[:, :, 1, :], op=OP.min
                )
                nc.vector.tensor_tensor(
                    nv[:, :, 1, :], cv[:, :, 0, :], cv[:, :, 1, :], op=OP.max
                )
                cur, nxt = nxt, cur
                d //= 2
            # cur = sorted ascending raw top-128 of the row

            # zap target values (by exact value match; sortedness not needed)
            if do_gather and cand_level >= 3:
                nc.vector.match_replace(
                    out=nxt[:], in_to_replace=tgt32[:, 0:8],
                    in_values=cur[:], imm_value=0.0,
                )
                nc.vector.match_replace(
                    out=cur[:], in_to_replace=tgt32[:, 8:16],
                    in_values=nxt[:], imm_value=0.0,
                )

            # keep the largest 100 surviving entries: kept prefix-count cum,
            # total K = cum[:, -1]; entry survives iff kept and cum >= K-99
            nc.vector.memset(zz[:], 0.0)
            nc.vector.tensor_scalar(
                zt[:], cur[:], 0.0, scalar2=None, op0=OP.is_gt
            )
            nc.vector.tensor_tensor_scan(
                out=ct[:], data0=zt[:], data1=zz[:], initial=0.0,
                op0=OP.add, op1=OP.add,
            )
            nc.vector.tensor_scalar_add(thr[:], ct[:, TAIL - 1:TAIL], -99.5)
            nc.vector.scalar_tensor_tensor(
                out=wt[:], in0=ct[:], scalar=thr[:, 0:1], in1=zt[:],
                op0=OP.is_ge, op1=OP.mult,
            )
            nc.vector.scalar_tensor_tensor(
                out=pt[:], in0=cur[:], scalar=1.0, in1=wt[:],
                op0=OP.mult, op1=OP.mult, accum_out=negrow[:],
            )

        # stacked cols = [sum_log, sum_margin, sum_neg]; matmul-reduce rows
        nc.vector.tensor_copy(stacked[0:BL, 2:3], negrow[:])
        nc.vector.memset(ones[:], 1.0)
        acc = psum_pool.tile([1, 3], f32, tag="acc")
        nc.tensor.matmul(acc[:], lhsT=ones[:], rhs=stacked[:], start=True, stop=True)
        nc.vector.tensor_copy(out_sb[:], acc[:])
        nc.sync.dma_start(out3[:, :], out_sb[:])

    nc.compile()
    return nc


def _get_nc():
    if "nc" not in _CACHE:
        _CACHE["nc"] = _build_nc()
    return _CACHE["nc"]


def make_in_maps(sparse_rep, target_ids):
    sp = np.ascontiguousarray(np.asarray(sparse_rep), dtype=np.float32)
    ids = np.asarray(target_ids)
    assert sp.shape == (B, V) and ids.shape == (B, T)
    in_maps = []
    r32 = np.arange(BL, dtype=np.int64)
    q32 = np.arange(128, dtype=np.int64) // 32       # q = p // 32
    p32 = np.arange(128, dtype=np.int64) % 32        # r = p % 32
    for i in range(N_CORES):
        rows = slice(BL * i, BL * (i + 1))
        idl = ids[rows].astype(np.int64)             # [32, 16]
        # off[p, g] = flat offset of (row p%32, target 4*(p//32)+g)
        offw = np.empty((128, GW), dtype=np.int64)
        for g in range(GW):
            offw[:, g] = p32 * V + idl[p32, 4 * q32 + g]
        in_maps.append({
            "sp": sp[rows],
            "off": offw.astype(np.int32),
        })
    return in_maps


def combine(parts):
    """parts: list of 8 [1,3] arrays -> (target_loss, margin_loss, negative_loss)"""
    acc = np.zeros(3, np.float64)
    for p in parts:
        acc += np.asarray(p, dtype=np.float64).reshape(3)
    target_loss = np.float32(-(acc[0] / (B * T)))
    margin_loss = np.float32(acc[1] / (B * T))
    negative_loss = np.float32(acc[2] / (B * TOP_K))
    return (target_loss, margin_loss, negative_loss)


def kernel(sparse_rep, target_ids):
    from concourse.bass_utils import run_bass_kernel_spmd

    nc = _get_nc()
    in_maps = make_in_maps(sparse_rep, target_ids)
    res = run_bass_kernel_spmd(nc, in_maps, list(range(N_CORES))).results
    return combine([r["out3"] for r in res])


# revision 33
# speedup vs baseline: 1.1343x; 1.1343x over previous
"""Trainium2 Bass kernel for DirectTargetLoss.

Computes, from sparse_rep [256, 128000] f32 and target_ids [256, 16] i64:
  target_loss   = -mean(log(gather(sparse_rep, target_ids) + 1e-8))
  margin_loss   = mean(relu(1 - gather(sparse_rep, target_ids)))
  negative_loss = mean(top_k(sparse_rep with target cols masked to -1e30, 100))

Sharding: data-parallel over the batch axis across 8 NeuronCores
(32 rows/core).  Per core:
  - the [32, 128000] shard is streamed into SBUF as 8 tiles of
    [128, 4000] f32 (each tile = 4 rows x 2 column-halves of 64000;
    each half spread over 16 partitions),
  - the gpsimd top-256 instruction runs per tile (8 "tokens" = 8
    half-rows; exact, sorted top-256 values per half),
  - per row, the top-128 of each half (sorted tails) are concatenated
    into a 256-candidate tile (the masked top-100 of the full row is
    always contained in these 240+ survivors),
  - the row's 16 target activations (gathered via 4 indirect DMAs of
    128 offsets each) are zapped out of the candidates by exact value
    match (match_replace); equal-value collisions can only swap equal
    contributions, so the top-100 sum is unchanged,
  - 13 rounds of max8 + match_replace extract the top 100 exactly,
  - the gathered targets also feed Ln / Relu activations with
    accumulation for the other two losses,
  - a ones-vector matmul reduces the per-row partials to [1, 3].
Host sums the 8 per-core [1,3] partials and normalizes.
"""

import numpy as np

B = 256
V = 128000
T = 16
TOP_K = 100
EPS = 1e-8
N_CORES = 8
BL = B // N_CORES          # 32 rows per core
RPT = 4                    # rows per topk call (2 halves each -> 8 tokens)
NT = BL // RPT             # 8 tiles per core
HALF = V // 2              # 64000
SEG = 16                   # partitions per half-row
F = HALF // SEG            # 4000 free elems per partition
KC = 256                   # topk k (per half-row)
TAIL = 128                 # kept tail per half
NC_CAND = 2 * TAIL         # 256 candidates per row
GW = T // 4                # 4 gather calls of 128 offsets

_CACHE = {}


def _build_nc(
    do_topk=True,
    do_cand=True,
    do_gather=True,
    load_engines=("sync",),
    loads_per_tile=1,
    cand_level=99,
    cand_rounds=13,
):
    from contextlib import ExitStack

    import concourse.bass as bass
    import concourse.tile as tile
    from concourse import bacc, mybir

    f32 = mybir.dt.float32
    i32 = mybir.dt.int32
    u32 = mybir.dt.uint32
    AF = mybir.ActivationFunctionType
    OP = mybir.AluOpType

    nc = bacc.Bacc("TRN2", target_bir_lowering=False, debug=False)

    sp = nc.dram_tensor("sp", [BL, V], f32, kind="ExternalInput")
    off = nc.dram_tensor("off", [128, GW], i32, kind="ExternalInput")
    out3 = nc.dram_tensor("out3", [1, 3], f32, kind="ExternalOutput")

    with tile.TileContext(nc) as tc, ExitStack() as ctx:
        cand_pool = ctx.enter_context(tc.tile_pool(name="cand", bufs=1))
        small_pool = ctx.enter_context(tc.tile_pool(name="small", bufs=1))
        psum_pool = ctx.enter_context(tc.tile_pool(name="psum", bufs=1, space="PSUM"))

        valsF = cand_pool.tile([BL, 2 * KC], f32, tag="valsF")
        b0 = cand_pool.tile([BL, TAIL], f32, tag="b0")
        b1 = cand_pool.tile([BL, TAIL], f32, tag="b1")
        zt = cand_pool.tile([BL, TAIL], f32, tag="zt")
        ct = cand_pool.tile([BL, TAIL], f32, tag="ct")
        zz = cand_pool.tile([BL, TAIL], f32, tag="zz")
        wt = cand_pool.tile([BL, TAIL], f32, tag="wt")
        pt = cand_pool.tile([BL, TAIL], f32, tag="pt")

        off_sb = small_pool.tile([128, GW], i32, tag="off_sb")
        tgtw = small_pool.tile([128, GW], f32, tag="tgtw")
        tgt32 = small_pool.tile([BL, T], f32, tag="tgt32")
        lnoutW = small_pool.tile([128, GW], f32, tag="lnoutW")
        mgoutW = small_pool.tile([128, GW], f32, tag="mgoutW")
        eps_t = small_pool.tile([128, 1], f32, tag="eps_t")
        thr = small_pool.tile([BL, 1], f32, tag="thr")
        negrow = small_pool.tile([BL, 1], f32, tag="negrow")
        stacked = small_pool.tile([128, 3], f32, tag="stacked")
        ones = small_pool.tile([128, 1], f32, tag="ones")
        out_sb = small_pool.tile([1, 3], f32, tag="out_sb")

        load_eng = [getattr(nc, e) for e in load_engines]

        nc.vector.memset(stacked[:], 0.0)
        nc.vector.memset(negrow[:], 0.0)

        # offsets for the gathers: off[p, g] targets (row p%32, t = 4*(p//32)+g)
        nc.sync.dma_start(off_sb[:], off[:, :])

        # --- big loads + per-half top-256 ---
        # tile c covers rows 4c..4c+3; token (j,h) = row 4c+j, col half h
        # sits on partitions 16*(2j+h) .. +15, 4000 contiguous elems each.
        dma_i = 0
        for c in range(NT):
            data = nc.alloc_sbuf_tensor(f"data{c}", [128, F], f32).ap()
            src = sp[RPT * c:RPT * (c + 1), :].rearrange(
                "j (h s f) -> (j h s) f", h=2, s=SEG
            )
            pchunk = 128 // loads_per_tile
            for li in range(loads_per_tile):
                eng = load_eng[dma_i % len(load_eng)]
                dma_i += 1
                pr = slice(pchunk * li, pchunk * (li + 1))
                eng.dma_start(data[pr, :], src[pr, :])

            if do_topk:
                tout = nc.alloc_sbuf_tensor(f"tout{c}", [128, 32], u32).ap()
                nc.gpsimd.topk(tout[:], data[:], tokens=8, vocab_size=HALF, k=KC)

                if do_cand:
                    # all 256 sorted-asc values per half -> row-major
                    # [4 rows, 2*256] (row r cols = [h0 asc | h1 asc])
                    dst_v = valsF[RPT * c:RPT * (c + 1), :].rearrange(
                        "j (h s f) -> j h s f", h=2, f=16
                    )
                    nc.scalar.dma_start(dst_v, tout[:, 0:16].bitcast(f32))

        # --- target gather: 4 indirect DMAs, 128 offsets each ---
        if do_gather:
            sp_flat = sp[:, :].rearrange("b (v one) -> (b v) one", one=1)
            for g in range(GW):
                nc.gpsimd.indirect_dma_start(
                    out=tgtw[:, g:g + 1],
                    out_offset=None,
                    in_=sp_flat,
                    in_offset=bass.IndirectOffsetOnAxis(
                        ap=off_sb[:, g:g + 1], axis=0
                    ),
                )
            # target_loss partial: sum(log(tgt + eps)); margin: sum(relu(1-tgt))
            nc.vector.memset(eps_t[:], EPS)
            nc.scalar.activation(
                lnoutW[:], tgtw[:], AF.Ln,
                bias=eps_t[:, 0:1], scale=1.0, accum_out=stacked[:, 0:1],
            )
            nc.scalar.activation(
                mgoutW[:], tgtw[:], AF.Relu,
                bias=1.0, scale=-1.0, accum_out=stacked[:, 1:2],
            )
            # row-major copy for the candidate zap: tgt32[r, 4q+g] =
            # tgtw[r+32q, g]
            for q in range(4):
                nc.scalar.dma_start(
                    tgt32[:, 4 * q:4 * (q + 1)], tgtw[32 * q:32 * (q + 1), :]
                )

        if do_topk and do_cand and cand_level >= 2:
            # --- top-128 of the union of the two sorted half-tails ---
            # A = half0 tail (asc), revB = half1 tail reversed (desc);
            # [A | revB] is bitonic, so hi_i = max(A_i, B_rev_i) is the
            # top-128 of the union (itself bitonic).
            a_view = valsF[:, KC - TAIL:KC]
            b_last = valsF[:, 2 * KC - 1:2 * KC]
            b_rev = bass.AP(
                b_last.tensor, b_last.offset,
                [list(b_last.ap[0]), [-1, TAIL]],
            )
            nc.vector.tensor_tensor(b0[:], a_view, b_rev, op=OP.max)

            # --- bitonic merge: 7 stages of paired min/max -> ascending ---
            cur, nxt = b0, b1
            d = TAIL // 2
            while d >= 1:
                cv = cur[:].rearrange("p (b two d) -> p b two d", two=2, d=d)
                nv = nxt[:].rearrange("p (b two d) -> p b two d", two=2, d=d)
                nc.vector.tensor_tensor(
                    nv[:, :, 0, :], cv[:, :, 0, :], cv[:, :, 1, :], op=OP.min
                )
                nc.vector.tensor_tensor(
                    nv[:, :, 1, :], cv[:, :, 0, :], cv[:, :, 1, :], op=OP.max
                )
                cur, nxt = nxt, cur
                d //= 2
            # cur = sorted ascending raw top-128 of the row

            # zap target values (by exact value match; sortedness not needed)
            if do_gather and cand_level >= 3:
                nc.vector.match_replace(
                    out=nxt[:], in_to_replace=tgt32[:, 0:8],
                    in_values=cur[:], imm_value=0.0,
                )
                nc.vector.match_replace(
                    out=cur[:], in_to_replace=tgt32[:, 8:16],
                    in_values=nxt[:], imm_value=0.0,
                )

            # keep the largest 100 surviving entries: kept prefix-count cum,
            # total K = cum[:, -1]; entry survives iff kept and cum >= K-99
            nc.vector.memset(zz[:], 0.0)
            nc.vector.tensor_scalar(
                zt[:], cur[:], 0.0, scalar2=None, op0=OP.is_gt
            )
            nc.vector.tensor_tensor_scan(
                out=ct[:], data0=zt[:], data1=zz[:], initial=0.0,
                op0=OP.add, op1=OP.add,
            )
            nc.vector.tensor_scalar_add(thr[:], ct[:, TAIL - 1:TAIL], -99.5)
            nc.vector.scalar_tensor_tensor(
                out=wt[:], in0=ct[:], scalar=thr[:, 0:1], in1=zt[:],
                op0=OP.is_ge, op1=OP.mult,
            )
            nc.vector.scalar_tensor_tensor(
                out=pt[:], in0=cur[:], scalar=1.0, in1=wt[:],
                op0=OP.mult, op1=OP.mult, accum_out=negrow[:],
            )

        # stacked cols = [sum_log, sum_margin, sum_neg]; matmul-reduce rows
        nc.vector.tensor_copy(stacked[0:BL, 2:3], negrow[:])
        nc.vector.memset(ones[:], 1.0)
        acc = psum_pool.tile([1, 3], f32, tag="acc")
        nc.tensor.matmul(acc[:], lhsT=ones[:], rhs=stacked[:], start=True, stop=True)
        nc.vector.tensor_copy(out_sb[:], acc[:])
        nc.sync.dma_start(out3[:, :], out_sb[:])

    nc.compile()
    return nc


def _get_nc():
    if "nc" not in _CACHE:
        _CACHE["nc"] = _build_nc()
    return _CACHE["nc"]


def make_in_maps(sparse_rep, target_ids):
    sp = np.ascontiguousarray(np.asarray(sparse_rep), dtype=np.float32)
    ids = np.asarray(target_ids)
    assert sp.shape == (B, V) and ids.shape == (B, T)
    in_maps = []
    r32 = np.arange(BL, dtype=np.int64)
    q32 = np.arange(128, dtype=np.int64) // 32       # q = p // 32
    p32 = np.arange(128, dtype=np.int64) % 32        # r = p % 32
    for i in range(N_CORES):
        rows = slice(BL * i, BL * (i + 1))
        idl = ids[rows].astype(np.int64)             # [32, 16]
        # off[p, g] = flat offset of (row p%32, target 4*(p//32)+g)
        offw = np.empty((128, GW), dtype=np.int64)
        for g in range(GW):
            offw[:, g] = p32 * V + idl[p32, 4 * q32 + g]
        in_maps.append({
            "sp": sp[rows],
            "off": offw.astype(np.int32),
        })
    return in_maps


def combine(parts):
    """parts: list of 8 [1,3] arrays -> (target_loss, margin_loss, negative_loss)"""
    acc = np.zeros(3, np.float64)
    for p in parts:
        acc += np.asarray(p, dtype=np.float64).reshape(3)
    target_loss = np.float32(-(acc[0] / (B * T)))
    margin_loss = np.float32(acc[1] / (B * T))
    negative_loss = np.float32(acc[2] / (B * TOP_K))
    return (target_loss, margin_loss, negative_loss)


def _get_runner():
    """Cached PJRT runner: jit/compile once, fast dispatch afterwards."""
    if "runner" in _CACHE:
        return _CACHE["runner"]

    import jax
    import jax.numpy as jnp  # noqa: F401
    from jax.sharding import Mesh, PartitionSpec
    from jax.experimental.shard_map import shard_map

    import concourse.mybir as mybir
    from concourse.bass2jax import (
        _bass_exec_p,
        install_neuronx_cc_hook,
        partition_id_tensor,
    )

    install_neuronx_cc_hook()
    nc = _get_nc()
    assert nc.dbg_addr is None
    partition_name = (
        nc.partition_id_tensor.name if nc.partition_id_tensor else None
    )

    in_names, out_names, out_avals, zero_shapes = [], [], [], []
    for alloc in nc.m.functions[0].allocations:
        if not isinstance(alloc, mybir.MemoryLocationSet):
            continue
        name = alloc.memorylocations[0].name
        if alloc.kind == "ExternalInput":
            if name != partition_name:
                in_names.append(name)
        elif alloc.kind == "ExternalOutput":
            out_names.append(name)
            shape = tuple(alloc.tensor_shape)
            dtype = mybir.dt.np(alloc.dtype)
            out_avals.append(jax.core.ShapedArray(shape, dtype))
            zero_shapes.append((shape, dtype))
    n_params = len(in_names)
    n_outs = len(out_names)
    all_names = list(in_names + out_names)
    if partition_name is not None:
        all_names.append(partition_name)
    all_names = tuple(all_names)
    donate = tuple(range(n_params, n_params + n_outs))

    def _body(*args):
        operands = list(args)
        if partition_name is not None:
            operands.append(partition_id_tensor())
        outs = _bass_exec_p.bind(
            *operands,
            out_avals=tuple(out_avals),
            in_names=all_names,
            out_names=tuple(out_names),
            lowering_input_output_aliases=(),
            sim_require_finite=True,
            sim_require_nnan=True,
            nc=nc,
        )
        return tuple(outs)

    devices = jax.devices()[:N_CORES]
    mesh = Mesh(np.asarray(devices), ("core",))
    sharded = jax.jit(
        shard_map(
            _body, mesh=mesh,
            in_specs=(PartitionSpec("core"),) * (n_params + n_outs),
            out_specs=(PartitionSpec("core"),) * n_outs,
            check_rep=False,
        ),
        donate_argnums=donate,
        keep_unused=True,
    )

    def run(in_maps):
        concat_in = [
            np.concatenate([np.asarray(m[name]) for m in in_maps], axis=0)
            for name in in_names
        ]
        concat_zeros = [
            np.zeros((N_CORES * s[0], *s[1:]), d) for (s, d) in zero_shapes
        ]
        out_arrs = sharded(*concat_in, *concat_zeros)
        return [
            {
                name: np.asarray(out_arrs[i]).reshape(
                    N_CORES, *out_avals[i].shape
                )[c]
                for i, name in enumerate(out_names)
            }
            for c in range(N_CORES)
        ]

    _CACHE["runner"] = run
    return run


def kernel(sparse_rep, target_ids):
    run = _get_runner()
    in_maps = make_in_maps(sparse_rep, target_ids)
    res = run(in_maps)
    return combine([r["out3"] for r in res])
